# revision 16
# baseline (speedup 1.0000x reference)
"""Binarized 3-layer MLP (B=8192, H=4096) on 8 Trainium2 NeuronCores.

Strategy: data-parallel over batch (1024 rows/core), weights replicated.
All matmul operands are exactly +-1, so the GEMMs are exact in fp8
(products +-1, fp32 PSUM accumulation). BatchNorm+binarize folds into an
integer threshold per output channel computed on the host; on-device it is
a single ScalarE Sign activation with per-partition bias 1 - T_o.

Layout is feature-major: activations in SBUF as [128 partitions, 32 chunks
x 1024 batch]. GEMMs run fp8e4 perf_mode=DoubleRow (256-deep contraction
per matmul, 2x bf16 throughput). Layer 1 is restructured into 4 passes
(batch-half x 8-chunk d-block) with an SBUF fp32 accumulator spill between
the two d-blocks, so the first o-tile needs only 1 MB of x (not 4 MB) and
the PE starts ~6 us earlier; layer-1 weight half-tiles are re-fetched for
the second batch half (+16 MB HBM, amortized). Weight DMAs alternate
between the two HWDGE queues (sync/scalar) to double weight-stream
bandwidth. Layers 2-3 are the plain 32 o-tile x 16 double-chunk x 2
batch-half accumulation. The 10-wide output layer uses 4-way PE column
tiling; its four partial-sum groups are copied PSUM->SBUF once and DMA'd
out in 4 parallel queue transfers, folded on the host.
"""

import numpy as np
import ml_dtypes

N_CORES = 8
B, H, L, NCOUT = 8192, 4096, 3, 10
BC = B // N_CORES          # batch per core
NT = H // 128              # 32 tiles of 128 along any H axis
BN_EPS = np.float32(1e-5)
TN_EPS = np.float32(1e-4)
HALF = BC // 2             # 512: one PSUM bank of fp32 per matmul
ND = H // 256              # 16 double-row chunks of 256 along contraction
NDH = ND // 2              # 8 chunks per d-block in the layer-1 passes

TRACE = False              # test harness may flip this for NTFF profiling
TRACE_DIR = None
LAST_EXEC_NS = None

_BUILD_CACHE = {}


def _split_multi_waits(nc):
    """walrus' CoreV3 codegen rejects instructions carrying more than one
    semaphore wait. Hoist all-but-one wait of any multi-wait instruction
    into standalone NoOps (same engine, placed immediately before)."""
    import bass_rust
    import concourse.mybir as mybir

    n = 0
    for f in nc.m.functions:
        for blk in f.blocks:
            out = []
            changed = False
            for inst in blk.instructions:
                si = inst.sync_info
                if si is not None and si.on_wait and len(si.on_wait) > 1:
                    waits = list(si.on_wait)
                    for w in waits[:-1]:
                        n += 1
                        nop = mybir.InstNoOp(name=f"waitsplit_{n}", ins=[], outs=[])
                        nop.engine = inst.engine
                        nop.sync_info = bass_rust.SyncInfo(on_wait=[w], on_update=[])
                        out.append(nop)
                    inst.sync_info = bass_rust.SyncInfo(
                        on_wait=[waits[-1]], on_update=list(si.on_update or [])
                    )
                    changed = True
                out.append(inst)
            if changed:
                blk.instructions = out
    return nc


def _build():
    if "nc" in _BUILD_CACHE:
        return _BUILD_CACHE["nc"]

    import concourse.bass as bass
    import concourse.mybir as mybir
    from concourse.tile import TileContext

    dt_w = mybir.dt.float8e4
    f32 = mybir.dt.float32

    wout_w = NT * NCOUT
    nc = bass.Bass()
    # x: 16 chunk tiles, each [128, h*1024 + j*512 + b] (batch-half-major)
    xin = nc.dram_tensor("x", [ND, 128, 2 * BC], dt_w, kind="ExternalInput")
    win = nc.dram_tensor("w", [L, NT, 128, H], dt_w, kind="ExternalInput")
    biasin = nc.dram_tensor("bias", [128, L * NT], f32, kind="ExternalInput")
    woutin = nc.dram_tensor("wout", [128, wout_w], dt_w, kind="ExternalInput")
    # 4 column-group partial sums, folded on the host
    outd = nc.dram_tensor("out", [4, NCOUT, BC], f32, kind="ExternalOutput")

    with TileContext(nc) as tc:
        with (
            tc.tile_pool(name="const", bufs=1) as constp,
            tc.tile_pool(name="acts", bufs=1) as actp,
            tc.tile_pool(name="wpool", bufs=4) as wp,
            tc.tile_pool(name="wpool0", bufs=8) as wp0,
            tc.tile_pool(name="psum", bufs=3, space="PSUM") as pp,
            tc.tile_pool(name="psumf", bufs=1, space="PSUM") as ppf,
            tc.tile_pool(name="outp", bufs=1) as op,
        ):
            # bias/wout ride the gpsimd SWDGE queue: small rows would clog the
            # HW queues that x and the weights need at startup
            bias_t = constp.tile([128, L * NT], f32, tag="bias")
            nc.gpsimd.dma_start(bias_t[:], biasin[:])
            wout_t = constp.tile([128, wout_w], dt_w, tag="wout")
            nc.gpsimd.dma_start(wout_t[:], woutin[:])

            plane0 = actp.tile([128, NT * BC], dt_w, tag="plane0")
            plane1 = actp.tile([128, NT * BC], dt_w, tag="plane1")
            planes = [plane0, plane1]
            # layer-1 accumulator spill: o-tile t at [:, t*1024:(t+1)*1024].
            # f16 is exact here: block partials are integers with |v| <= 2048
            # (2048 products of +-1), within f16's 11-bit significand.
            acc = actp.tile([128, NT * BC], mybir.dt.float16, tag="acc")

            xtiles = [
                actp.tile([128, 2 * BC], dt_w, tag=f"xt{d}", name=f"xt{d}")
                for d in range(ND)
            ]

            # --- DMA layout: ALL weights ride the sync HWDGE queue (the sync
            # engine runs no compute, so its dma_starts issue far ahead,
            # pool-depth limited). ALL x pieces ride the scalar HWDGE queue
            # and are emitted up-front, before any ScalarE compute can gate
            # them. Exception: the first d-block's pieces alternate onto sync
            # too (interleaved with the first weight tiles) so the ~2 MB the
            # first o-tile needs lands at the aggregate ~350 GB/s rate. ---
            wt_first = wp0.tile([128, H // 2], dt_w, tag="wl0", name="wt_first")
            # split so the very first matmul only waits for a 32 KB sliver
            nc.sync.dma_start(wt_first[:, 0:256], win[0, 0, :, 0:256])
            nc.sync.dma_start(wt_first[:, 256 : H // 2], win[0, 0, :, 256 : H // 2])

            def xdma(eng, d, h):
                eng.dma_start(
                    xtiles[d][:, h * BC : (h + 1) * BC],
                    xin[d, :, h * BC : (h + 1) * BC],
                )

            # blk0 pieces in consumption order: h0 d0..7, then h1 d0..7
            p0 = [(d, 0) for d in range(NDH)] + [(d, 1) for d in range(NDH)]
            for i, (d, h) in enumerate(p0):
                xdma(nc.scalar if i % 2 == 0 else nc.sync, d, h)
            # blk1 pieces are needed only after ~110 us; drip them into the
            # scalar queue inside the blk0 loop, BEHIND the odd weight
            # fetches (emitting them up-front would park 2 MB ahead of the
            # weights in the queue FIFO and starve the PE)
            x_rest = [(d, 0) for d in range(NDH, ND)] + [(d, 1) for d in range(NDH, ND)]

            def wdma(tile_ap, src_ap):
                nc.sync.dma_start(tile_ap, src_ap)

            # --- layer 1: two full-batch passes over d-blocks of 8 chunks ---
            # Weight demand here is ~148 GB/s (a 256 KB half-tile every
            # 1.73 us) — more than one HWDGE queue sustains. Alternate the
            # fetches over both queues with a +2-tile prefetch offset so the
            # scalar-engine-issued ones leave ~2 tiles before they're needed
            # (the scalar engine's dma_starts queue behind its copies/signs).
            l0_tiles = [wt_first] + [
                wp0.tile([128, H // 2], dt_w, tag="wl0", name=f"wl0_{i}")
                for i in range(1, 2 * NT)
            ]

            def l0_fetch(i):
                if not 1 <= i < 2 * NT:
                    return
                blk, t = divmod(i, NT)
                eng = nc.sync if i % 2 == 0 else nc.scalar
                eng.dma_start(
                    l0_tiles[i][:],
                    win[0, t, :, blk * (H // 2) : (blk + 1) * (H // 2)],
                )

            l0_fetch(1)
            l0_fetch(2)
            for blk in (0, 1):
                for t in range(NT):
                    wt = l0_tiles[blk * NT + t]
                    l0_fetch(blk * NT + t + 3)
                    if blk == 0 and 8 <= t < 8 + len(x_rest):
                        xdma(nc.scalar, *x_rest[t - 8])
                    ps = pp.tile([128, BC], f32, tag="ps")
                    w3 = wt[:].rearrange("p (d j m) -> p d j m", d=NDH, j=2)
                    for h in (0, 1):
                        for d8 in range(NDH):
                            rhs = xtiles[blk * NDH + d8][:].rearrange(
                                "p (hh j b) -> p hh j b", hh=2, j=2
                            )[:, h]
                            nc.tensor.matmul(
                                ps[:, h * HALF : (h + 1) * HALF], w3[:, d8], rhs,
                                start=(d8 == 0), stop=(d8 == NDH - 1),
                                perf_mode=mybir.MatmulPerfMode.DoubleRow,
                            )
                    acc_ap = acc[:, t * BC : (t + 1) * BC]
                    if blk == 0:
                        nc.scalar.copy(acc_ap, ps[:])
                    else:
                        # fold the block-0 partial back in, then threshold
                        nc.vector.tensor_add(ps[:], ps[:], acc_ap)
                        bias_ap = bias_t[:, t : t + 1]
                        nc.scalar.sign(
                            plane1[:, t * BC : (t + 1) * BC], ps[:], bias=bias_ap,
                        )

            # --- layers 2..3: plain o-tile loop, both halves per psum tile.
            # The 10-channel output layer's column-tiled quads are interleaved
            # into layer 3: the quad for chunks 4k..4k+3 is emitted right
            # after the sign of o-tile 4k+3, so only the last quad remains
            # after the final sign instead of the whole output layer. ---
            psf = ppf.tile([128, BC], f32, tag="psf", name="psf")

            def out_quad(k, src):
                for c in range(4 * k, 4 * k + 4):
                    g = c % 4
                    lhsT = wout_t[:, c * NCOUT : (c + 1) * NCOUT]
                    a0 = src[:, c * BC : c * BC + HALF]
                    a1 = src[:, c * BC + HALF : (c + 1) * BC]
                    nc.tensor.matmul(
                        psf[32 * g : 32 * g + NCOUT, 0:HALF], lhsT, a0,
                        start=(c < 4), stop=(c >= NT - 4), tile_position=(0, 32 * g),
                    )
                    nc.tensor.matmul(
                        psf[32 * g : 32 * g + NCOUT, HALF:BC], lhsT, a1,
                        start=(c < 4), stop=(c >= NT - 4), tile_position=(0, 32 * g),
                    )

            cur = 1
            for l in range(1, L):
                src, dst = planes[cur], planes[1 - cur]
                src3 = src[:].rearrange("p (c b) -> p c b", c=NT)
                for t in range(NT):
                    wt = wp.tile([128, H], dt_w, tag="wt")
                    wdma(wt[:], win[l, t])
                    ps = pp.tile([128, BC], f32, tag="ps")
                    w3 = wt[:].rearrange("p (d j m) -> p d j m", d=ND, j=2)
                    for d in range(ND):
                        lhsT = w3[:, d]
                        a0 = src3[:, 2 * d : 2 * d + 2, 0:HALF]
                        a1 = src3[:, 2 * d : 2 * d + 2, HALF:BC]
                        nc.tensor.matmul(
                            ps[:, 0:HALF], lhsT, a0,
                            start=(d == 0), stop=(d == ND - 1),
                            perf_mode=mybir.MatmulPerfMode.DoubleRow,
                        )
                        nc.tensor.matmul(
                            ps[:, HALF:BC], lhsT, a1,
                            start=(d == 0), stop=(d == ND - 1),
                            perf_mode=mybir.MatmulPerfMode.DoubleRow,
                        )
                    bias_ap = bias_t[:, l * NT + t : l * NT + t + 1]
                    if l == L - 1 and t == NT - 1:
                        # split the very last Sign so the last quad's
                        # chunk-31 matmuls unblock half a Sign earlier
                        nc.scalar.sign(
                            dst[:, t * BC : t * BC + HALF], ps[:, 0:HALF],
                            bias=bias_ap,
                        )
                        nc.scalar.sign(
                            dst[:, t * BC + HALF : (t + 1) * BC], ps[:, HALF:BC],
                            bias=bias_ap,
                        )
                    else:
                        nc.scalar.sign(dst[:, t * BC : (t + 1) * BC], ps[:], bias=bias_ap)
                    # quad k is emitted one o-tile AFTER sign(4k+3) so the
                    # in-order PE stream never waits on a just-issued sign
                    if l == L - 1 and t >= 5 and (t - 5) % 4 == 0:
                        out_quad((t - 5) // 4, dst)
                cur = 1 - cur
            out_quad(NT // 4 - 1, planes[cur])
            # one PSUM->SBUF copy, then the 4 group partials leave on both
            # HWDGE queues in parallel; the host folds the 4 partials.
            out_s = op.tile([128, BC], f32, tag="outs")
            nc.scalar.copy(out_s[:], psf[:])
            for g, eng in enumerate([nc.sync, nc.scalar, nc.sync, nc.scalar]):
                eng.dma_start(outd[g], out_s[32 * g : 32 * g + NCOUT, :])

    _split_multi_waits(nc)
    _BUILD_CACHE["nc"] = nc
    return nc


def _thresholds(bn_gamma, bn_beta, bn_mean, bn_var):
    """Per-channel even-integer threshold T with sign(BN(y)) = +1 <=> y >= T,
    mirroring the reference's fp32 arithmetic. gamma>0 so BN is increasing."""
    arg = (bn_var.astype(np.float32) + BN_EPS).astype(np.float32)
    rs = (1.0 / np.sqrt(arg.astype(np.float64))).astype(np.float32)
    y = np.arange(-H, H + 1, 2, dtype=np.float32)[:, None]  # [4097, 1]
    T = np.empty((L, H), np.float32)
    for l in range(L):
        z = ((y - bn_mean[l]) * rs[l]) * bn_gamma[l] + bn_beta[l]
        nz = z >= 0
        first = nz.argmax(axis=0)
        anyt = nz.any(axis=0)
        T[l] = np.where(anyt, -H + 2.0 * first, H + 2.0)
    return T


def kernel(x, W, Wout, bn_gamma, bn_beta, bn_mean, bn_var, tn_w, tn_b, tn_m, tn_v):
    global LAST_EXEC_NS
    from concourse.bass_utils import run_bass_kernel_spmd

    x = np.asarray(x, dtype=np.float32)
    W = np.asarray(W, dtype=np.float32)
    Wout = np.asarray(Wout, dtype=np.float32)
    bn_gamma = np.asarray(bn_gamma, dtype=np.float32)
    bn_beta = np.asarray(bn_beta, dtype=np.float32)
    bn_mean = np.asarray(bn_mean, dtype=np.float32)
    bn_var = np.asarray(bn_var, dtype=np.float32)

    np_dt = ml_dtypes.float8_e4m3

    # --- host prep: binarize + lay out ---
    xb = np.where(x.reshape(B, H) >= np.float32(0.5), 1.0, -1.0).astype(np_dt)
    xb = np.ascontiguousarray(xb.T)  # [H, B] feature-major

    Ws = np.where(W >= 0, 1.0, -1.0).astype(np_dt)  # [L, O, H]
    # w_dev[l, t, k, d*256 + j*128 + m] = Ws[l, t*128+m, (2d+j)*128+k]
    w_dev = np.ascontiguousarray(
        Ws.reshape(L, NT, 128, ND, 2, 128)
        .transpose(0, 1, 5, 3, 4, 2)
        .reshape(L, NT, 128, H)
    )

    T = _thresholds(bn_gamma, bn_beta, bn_mean, bn_var)
    # bias[p, l*NT+t] = 1 - T[l, t*128+p]
    bias_host = np.ascontiguousarray(
        (np.float32(1.0) - T).reshape(L, NT, 128).transpose(2, 0, 1).reshape(128, L * NT)
    )

    WoS = np.where(Wout >= 0, 1.0, -1.0).astype(np_dt)  # [10, H]
    # wout[k, c*10+j] = WoS[j, c*128+k]
    wout_host = np.ascontiguousarray(
        WoS.reshape(NCOUT, NT, 128).transpose(2, 1, 0).reshape(128, NT * NCOUT)
    )

    nc = _build()
    in_maps = []
    for core in range(N_CORES):
        sl = slice(core * BC, (core + 1) * BC)
        # half/pair-major: xc[d, p, h*1024 + j*512 + b] = xb[(2d+j)*128+p, h*512+b]
        xc = np.ascontiguousarray(
            xb[:, sl].reshape(ND, 2, 128, 2, HALF)   # d, j, p, h, b
            .transpose(0, 2, 3, 1, 4)                # d, p, h, j, b
            .reshape(ND, 128, 2 * BC)
        )
        in_maps.append(
            {"x": xc, "w": w_dev, "bias": bias_host, "wout": wout_host}
        )

    kwargs = {}
    if TRACE:
        kwargs = {"trace": True, "tmpdir": TRACE_DIR}
    # the first device open occasionally hits a transient
    # NRT_EXEC_UNIT_UNRECOVERABLE; a retry has always recovered it
    import time

    last_exc = None
    for attempt in range(3):
        try:
            res = run_bass_kernel_spmd(nc, in_maps, list(range(N_CORES)), **kwargs)
            break
        except Exception as exc:  # noqa: BLE001
            last_exc = exc
            time.sleep(5 * (attempt + 1))
    else:
        raise last_exc
    LAST_EXEC_NS = res.exec_time_ns

    out_int = np.concatenate(
        [
            np.asarray(res.results[c]["out"], dtype=np.float32).sum(axis=0).T
            for c in range(N_CORES)
        ],
        axis=0,
    )  # [B, 10] exact even integers

    rs_t = np.float32(1.0 / np.sqrt(np.float64(np.float32(tn_v) + TN_EPS)))
    out = ((out_int - np.float32(tn_m)) * rs_t) * np.float32(tn_w) + np.float32(tn_b)
    return out.astype(np.float32)
